# revision 1
# baseline (speedup 1.0000x reference)
"""Trainium2 Bass kernel for nn_Net_3152505995417 (gnn_message_passing).

Closed-form reformulation: with T the incidence matrix of a simple graph,
  node conv:  (T diag(d) T^T) * adj_v  ==  A with A[i,j] = d[edge(i,j)], 0 diag
  edge conv:  M = (T^T diag(dv) T) * adj_e has M[e,f] = dv[shared node],
              col-max(f=(k,l)) = max(dv[k], dv[l], 0)   (complete graph),
              row e=(i,j) of (M/colmax) @ G = dv_i*(S_i - Gn_e) + dv_j*(S_j - Gn_e)
              with Gn = G / (colmax + eps), S = T @ Gn.
So the E x E matrix is never materialized: everything lives in a dense
[N, N] node-pair layout (slots (i,j) and (j,i) both carry edge {i,j};
diagonal slots are zero). Gathers/scatters become row/column broadcasts
(PE ones-matmuls) and free-dim row-sums of [116, 116] tiles.

All inputs arrive in one packed [128, 1016] slab (two DMAs); the full
model runs replicated on each of the 8 NeuronCores (total work is a few
hundred KB — replication beats collective latency); core 0's output is
returned.
"""

import numpy as np

N = 116
E = N * (N - 1) // 2
HID = 64
EDIM = 5
OUT = 4
ENC = HID + N // 2
EPS = 1e-10

# packed slab column offsets
C_EA = 0                 # [0:116, 0:580]   ea dense, k-major (f = k*N + j)
C_SVEC = 1016            # [0, 1016:1056]   p1|p2|be|We.flat
C_ENCT = 580             # [0:122, 580:696]
C_WENC = 696             # [0:122, 696:760]
C_W1 = 760               # [0:64]
C_W2 = 824
C_WL = 888               # [0:64, 888:892]
C_MASK = 892             # [0:116, 892:1008]
C_BENC = 1008
C_B1 = 1009
C_B2 = 1010
C_PET = 1011
C_BL = 1012              # [0:4]
SLAB_W = 1056
SPLIT = 580              # DMA A = cols [0:580], DMA B = cols [580:1016]

# plane-boundary chunking of the 5*116 edge-conv slab (PE N<=512, PSUM bank)
CH = [(0, 232), (232, 580)]

_CACHE = {}


def _split_excess_waits(nc, mybir, max_waits=1):
    """Workaround: this walrus build accepts only one sync-wait per
    instruction (setupSyncWait: "Too many sync wait commands"). Move excess
    waits onto chained NoOps on the same engine immediately before the
    instruction; sequencer semantics are unchanged."""
    for fn in nc.m.functions:
        for blk in fn.blocks:
            insts = blk.instructions
            new, changed = [], False
            for ins in insts:
                si = ins.sync_info
                waits = list(si.on_wait) if si is not None else []
                if len(waits) > max_waits:
                    while len(waits) > max_waits:
                        chunk, waits = waits[:1], waits[1:]
                        nop = mybir.InstNoOp(
                            name=nc.get_next_instruction_name(),
                            engine=ins.engine,
                            sync_info=mybir.SyncInfo(on_wait=chunk, on_update=[]),
                            bass_nofuse=True,
                        )
                        new.append(nop)
                    si.on_wait = waits
                    changed = True
                new.append(ins)
            if changed:
                blk.instructions = new


def _build():
    import concourse.bass as bass
    import concourse.tile as tile
    from concourse import mybir

    f32 = mybir.dt.float32
    A = mybir.AluOpType
    Relu = mybir.ActivationFunctionType.Relu

    nc = bass.Bass("TRN2", target_bir_lowering=False, num_devices=8)

    slabA_d = nc.declare_dram_parameter("slabA", [128, SPLIT], f32, isOutput=False)
    slabB_d = nc.declare_dram_parameter(
        "slabB", [128, SLAB_W - SPLIT], f32, isOutput=False
    )
    out_d = nc.declare_dram_parameter("out", [OUT, 1], f32, isOutput=True)

    with tile.TileContext(nc) as tc:
        with (
            tc.tile_pool(name="sb", bufs=1) as sb,
            tc.tile_pool(name="pm", bufs=2) as pm,
            tc.tile_pool(name="ps", bufs=3, space="PSUM") as ps,
            tc.tile_pool(name="ps2", bufs=2, space="PSUM") as ps2,
        ):
            tA = sb.tile([128, SPLIT], f32, tag="tA")
            tB = sb.tile([128, SLAB_W - SPLIT], f32, tag="tB")
            # four engines copy in parallel (each dma_start is a blocking
            # sequencer copy at ~80GB/s on this build)
            nc.gpsimd.dma_start(out=tA[:, 0:348], in_=slabA_d[:, 0:348])
            nc.scalar.dma_start(out=tA[:, 348:SPLIT], in_=slabA_d[:, 348:SPLIT])
            nc.sync.dma_start(out=tB[:], in_=slabB_d[:])

            def B(c):
                return c - SPLIT

            ea = tA[0:N, 0:EDIM * N]
            svec = tB[0:1, B(C_SVEC):B(C_SVEC) + 40]
            encT = tB[0:ENC, B(C_ENCT):B(C_ENCT) + N]
            Wenc = tB[0:ENC, B(C_WENC):B(C_WENC) + HID]
            W1 = tB[0:HID, B(C_W1):B(C_W1) + HID]
            W2 = tB[0:HID, B(C_W2):B(C_W2) + HID]
            Wl = tB[0:HID, B(C_WL):B(C_WL) + OUT]
            mask = tB[0:N, B(C_MASK):B(C_MASK) + N]
            benc = tB[0:HID, B(C_BENC):B(C_BENC) + 1]
            b1 = tB[0:HID, B(C_B1):B(C_B1) + 1]
            b2 = tB[0:HID, B(C_B2):B(C_B2) + 1]
            peT = tB[0:HID, B(C_PET):B(C_PET) + 1]
            bl = tB[0:OUT, B(C_BL):B(C_BL) + 1]

            ones_row = sb.tile([1, N], f32, tag="ones_row")
            nc.vector.memset(ones_row[:], 1.0)
            ones_col = sb.tile([N, 1], f32, tag="ones_col")
            nc.vector.memset(ones_col[:], 1.0)
            warm = sb.tile([1, 1], f32, tag="warm")
            nc.scalar.activation(warm[:], ones_row[:, 0:1], Relu)
            zerosT = sb.tile([N, N], f32, tag="zerosT")
            nc.vector.memset(zerosT[:], 0.0)

            # ---- broadcast the small row-vector params to all partitions ----
            svecB_ps = ps.tile([N, 40], f32, tag="ps")
            nc.tensor.matmul(svecB_ps[:], ones_row[:], svec, start=True, stop=True)
            svecB = sb.tile([N, 40], f32, tag="svecB")
            nc.vector.tensor_copy(svecB[:], svecB_ps[:])
            p1B = svecB[:, 0:5]
            p2B = svecB[:, 5:10]
            beB = svecB[:, 10:15]
            # We[k, m] at column 15 + k*5 + m

            # ---- x = enc @ W_enc + b_enc  (kept transposed: [HID, N]) ----
            xT_ps = ps.tile([HID, N], f32, tag="ps")
            nc.tensor.matmul(xT_ps[:], Wenc, encT, start=True, stop=True)
            xT = sb.tile([HID, N], f32, tag="xT")
            nc.vector.tensor_scalar_add(xT[:], xT_ps[:], benc)

            # ---- A1 = d1 (dense pair layout; diag slots already zero) ----
            d1 = sb.tile([N, N], f32, tag="d1")
            nc.vector.tensor_scalar_mul(d1[:], ea[:, 0:N], p1B[:, 0:1])
            for k in range(1, EDIM):
                nc.vector.scalar_tensor_tensor(
                    d1[:], ea[:, k * N:(k + 1) * N], p1B[:, k:k + 1], d1[:],
                    A.mult, A.add,
                )

            # ---- node conv 1: x1T = relu((A1 @ (x @ W1) + b1)^T) ----
            xW1_ps = ps.tile([N, HID], f32, tag="ps")
            nc.tensor.matmul(xW1_ps[:], xT[:], W1, start=True, stop=True)
            xW1 = sb.tile([N, HID], f32, tag="xW1")
            nc.vector.tensor_copy(xW1[:], xW1_ps[:])
            x1T_ps = ps.tile([HID, N], f32, tag="ps")
            nc.tensor.matmul(x1T_ps[:], xW1[:], d1[:], start=True, stop=True)
            x1T = sb.tile([HID, N], f32, tag="x1T")
            nc.scalar.activation(x1T[:], x1T_ps[:], Relu, bias=b1)

            # ---- dv = x1 @ pe^T, as row [1,N] and column [N,1] ----
            dvr_ps = ps.tile([1, N], f32, tag="ps")
            nc.tensor.matmul(dvr_ps[:], peT, x1T[:], start=True, stop=True)
            dv_row = sb.tile([1, N], f32, tag="dv_row")
            nc.vector.tensor_copy(dv_row[:], dvr_ps[:])
            dv_rowE = sb.tile([1, N], f32, tag="dv_rowE")
            nc.vector.tensor_scalar_add(dv_rowE[:], dvr_ps[:], EPS)
            dvT_ps = ps.tile([N, 1], f32, tag="ps")
            nc.tensor.matmul(dvT_ps[:], x1T[:], peT, start=True, stop=True)
            dvTe = sb.tile([N, 1], f32, tag="dvTe")
            nc.vector.tensor_scalar_add(dvTe[:], dvT_ps[:], EPS)
            dvROW_ps = ps.tile([N, N], f32, tag="ps")
            # dvROWe[i,j] = dv_j + eps (the +eps rides along; max is shift-
            # invariant so cmeps = max(dv_i+eps, dv_j+eps, eps))
            nc.tensor.matmul(dvROW_ps[:], ones_row[:], dv_rowE[:], start=True, stop=True)

            # negsumdv[i,j] = -(dv_i + dv_j) (up to 2eps, far below f32 ulp)
            negsumdv = sb.tile([N, N], f32, tag="negsumdv")
            nc.vector.tensor_scalar(
                negsumdv[:], dvROW_ps[:], dvTe[:, 0:1], -1.0, A.add, A.mult
            )
            cmeps = sb.tile([N, N], f32, tag="cmeps")
            nc.vector.tensor_scalar(
                cmeps[:], dvROW_ps[:], dvTe[:, 0:1], EPS, A.max, A.max
            )
            nc.vector.reciprocal(cmeps[:], cmeps[:])

            # ---- edge conv (plane-major slabs, f = m*N + j) ----
            eR = sb.tile([N, EDIM * N], f32, tag="eR")
            nc.scalar.activation(eR[:], ea, Relu)

            # G[:, (m,j)] = sum_k eR_k[:, j] * We[k, m]; eR_k broadcast along
            # m (outer step-0), We row broadcast along j (inner step-0).
            def eRk_b(k):
                return eR[:, k * N:(k + 1) * N][:, None, :].to_broadcast(
                    [N, EDIM, N]
                )

            def WeB_b(k):
                return svecB[:, 15 + k * 5:15 + k * 5 + 5][:, :, None].to_broadcast(
                    [N, EDIM, N]
                )

            G = sb.tile([N, EDIM * N], f32, tag="G")
            G3 = G[:].rearrange("p (m j) -> p m j", m=EDIM)
            Gt = sb.tile([N, EDIM * N], f32, tag="Gt")
            Gt3 = Gt[:].rearrange("p (m j) -> p m j", m=EDIM)
            Gg = sb.tile([N, EDIM * N], f32, tag="Gg")
            Gg3 = Gg[:].rearrange("p (m j) -> p m j", m=EDIM)
            nc.vector.tensor_tensor(G3, eRk_b(0), WeB_b(0), A.mult)
            for k in (1, 2, 3):
                nc.vector.tensor_tensor(Gt3, eRk_b(k), WeB_b(k), A.mult)
                nc.vector.tensor_tensor(G3, G3, Gt3, A.add)
            nc.gpsimd.tensor_tensor(Gg3, eRk_b(4), WeB_b(4), A.mult)
            nc.vector.tensor_tensor(G3, G3, Gg3, A.add)

            # Gn_m = G_m * (1/cmeps); accum_out gives S[:, m] = rowsum(Gn_m)
            # for free in the same instruction
            Gn = sb.tile([N, EDIM * N], f32, tag="Gn")
            S_all = sb.tile([N, EDIM], f32, tag="S_all")
            for m in range(EDIM):
                sl = slice(m * N, (m + 1) * N)
                nc.vector.scalar_tensor_tensor(
                    Gn[:, sl], G[:, sl], 0.0, cmeps[:], A.add, A.mult,
                    accum_out=S_all[:, m:m + 1],
                )
            # t1b[:, m] = dv_i * S[i, m] + be_m
            t1b = sb.tile([N, EDIM], f32, tag="t1b")
            nc.vector.scalar_tensor_tensor(
                t1b[:], S_all[:], dvTe[:, 0:1], beB, A.mult, A.add
            )

            # q = Gn * negsumdv (broadcast along m) -- on GpSimd, per chunk
            q = sb.tile([N, EDIM * N], f32, tag="q")
            for c0, c1 in CH:
                nm = (c1 - c0) // N
                nc.gpsimd.tensor_tensor(
                    q[:, c0:c1].rearrange("p (m j) -> p m j", m=nm),
                    Gn[:, c0:c1].rearrange("p (m j) -> p m j", m=nm),
                    negsumdv[:, None, :].to_broadcast([N, nm, N]),
                    A.mult,
                )

            # S as rows (colsum of symmetric Gn), then U[i, (m,j)] = dv_j*S[j,m]
            z = sb.tile([N, EDIM * N], f32, tag="z")
            for c0, c1 in CH:
                w = c1 - c0
                nm = w // N
                Srow_ps = ps2.tile([1, w], f32, tag="psrow")
                nc.tensor.matmul(
                    Srow_ps[:], ones_col[:], Gn[:, c0:c1], start=True, stop=True
                )
                u = pm.tile([1, w], f32, tag="u")
                nc.vector.tensor_tensor(
                    u[:].rearrange("p (m j) -> p m j", m=nm),
                    dv_row[:, None, :].to_broadcast([1, nm, N]),
                    Srow_ps[:].rearrange("p (m j) -> p m j", m=nm),
                    A.mult,
                )
                U_ps = ps2.tile([N, w], f32, tag="psU")
                nc.tensor.matmul(U_ps[:], ones_row[:], u[:], start=True, stop=True)
                nc.vector.tensor_tensor(z[:, c0:c1], q[:, c0:c1], U_ps[:], A.add)

            # e2_m = relu(z_m + t1b_m): split across ACT and DVE
            e2 = sb.tile([N, EDIM * N], f32, tag="e2")
            for m in range(EDIM):
                sl = slice(m * N, (m + 1) * N)
                if m in (0, 1, 2):
                    nc.scalar.activation(
                        e2[:, sl], z[:, sl], Relu, bias=t1b[:, m:m + 1]
                    )
                else:
                    nc.vector.scalar_tensor_tensor(
                        e2[:, sl], z[:, sl], t1b[:, m:m + 1], zerosT[:],
                        A.add, A.max,
                    )

            # ---- A2 = (e2 @ p2^T) * mask ----
            d2 = sb.tile([N, N], f32, tag="d2")
            nc.vector.tensor_scalar_mul(d2[:], e2[:, 0:N], p2B[:, 0:1])
            for m in range(1, EDIM):
                nc.vector.scalar_tensor_tensor(
                    d2[:], e2[:, m * N:(m + 1) * N], p2B[:, m:m + 1], d2[:],
                    A.mult, A.add,
                )
            A2 = sb.tile([N, N], f32, tag="A2")
            nc.vector.tensor_tensor(A2[:], d2[:], mask, A.mult)

            # ---- node conv 2 (no relu) + mean pool + head ----
            xW2_ps = ps.tile([N, HID], f32, tag="ps")
            nc.tensor.matmul(xW2_ps[:], x1T[:], W2, start=True, stop=True)
            xW2 = sb.tile([N, HID], f32, tag="xW2")
            nc.vector.tensor_copy(xW2[:], xW2_ps[:])
            x2T_ps = ps.tile([HID, N], f32, tag="ps")
            nc.tensor.matmul(x2T_ps[:], xW2[:], A2[:], start=True, stop=True)
            red = sb.tile([HID, 1], f32, tag="red")
            nc.vector.tensor_reduce(red[:], x2T_ps[:], mybir.AxisListType.X, A.add)
            pooledT = sb.tile([HID, 1], f32, tag="pooledT")
            nc.vector.tensor_scalar(
                pooledT[:], red[:], 1.0 / N, b2, A.mult, A.add
            )
            outT_ps = ps.tile([OUT, 1], f32, tag="ps")
            nc.tensor.matmul(outT_ps[:], Wl, pooledT[:], start=True, stop=True)
            out_sb = sb.tile([OUT, 1], f32, tag="out_sb")
            nc.vector.tensor_scalar_add(out_sb[:], outT_ps[:], bl)
            nc.sync.dma_start(out=out_d[:], in_=out_sb[:])

    _split_excess_waits(nc, mybir)
    return nc


def _prep_inputs(inputs):
    ei = np.asarray(inputs["edge_index"][0], dtype=np.int64)
    ej = np.asarray(inputs["edge_index"][1], dtype=np.int64)
    ea = np.asarray(inputs["edge_attr"], dtype=np.float32)

    ea_dense = np.zeros((N, EDIM, N), dtype=np.float32)
    ea_dense[ei, :, ej] = ea
    ea_dense[ej, :, ei] = ea

    slab = np.zeros((128, SLAB_W), dtype=np.float32)
    slab[0:N, 0:EDIM * N] = ea_dense.reshape(N, EDIM * N)
    slab[0, C_SVEC:C_SVEC + 40] = np.concatenate(
        [
            np.asarray(inputs["p1"], dtype=np.float32).reshape(-1),
            np.asarray(inputs["p2"], dtype=np.float32).reshape(-1),
            np.asarray(inputs["be"], dtype=np.float32).reshape(-1),
            np.asarray(inputs["We"], dtype=np.float32).reshape(-1),
        ]
    )
    slab[0:ENC, C_ENCT:C_ENCT + N] = np.asarray(
        inputs["encoding_raw"], dtype=np.float32
    ).T
    slab[0:ENC, C_WENC:C_WENC + HID] = np.asarray(inputs["W_enc"], dtype=np.float32)
    slab[0:HID, C_W1:C_W1 + HID] = np.asarray(inputs["W1"], dtype=np.float32)
    slab[0:HID, C_W2:C_W2 + HID] = np.asarray(inputs["W2"], dtype=np.float32)
    slab[0:HID, C_WL:C_WL + OUT] = np.asarray(inputs["Wl"], dtype=np.float32)
    slab[0:N, C_MASK:C_MASK + N] = 1.0 - np.eye(N, dtype=np.float32)
    slab[0:HID, C_BENC] = np.asarray(inputs["b_enc"], dtype=np.float32).reshape(-1)
    slab[0:HID, C_B1] = np.asarray(inputs["b1"], dtype=np.float32).reshape(-1)
    slab[0:HID, C_B2] = np.asarray(inputs["b2"], dtype=np.float32).reshape(-1)
    slab[0:HID, C_PET] = np.asarray(inputs["pe"], dtype=np.float32).reshape(-1)
    slab[0:OUT, C_BL] = np.asarray(inputs["bl"], dtype=np.float32).reshape(-1)

    return {
        "slabA": np.ascontiguousarray(slab[:, 0:SPLIT]),
        "slabB": np.ascontiguousarray(slab[:, SPLIT:SLAB_W]),
    }


def kernel(**inputs) -> np.ndarray:
    import sys

    if "/opt/trn_rl_repo" not in sys.path:
        sys.path.insert(0, "/opt/trn_rl_repo")
    from concourse.bass_utils import run_bass_kernel_spmd

    if "nc" not in _CACHE:
        _CACHE["nc"] = _build()
    nc = _CACHE["nc"]

    in_map = _prep_inputs(inputs)
    res = run_bass_kernel_spmd(
        nc, [in_map] * 8, core_ids=list(range(8)), trace=False
    )
    return np.asarray(res.results[0]["out"], dtype=np.float32).reshape(1, OUT)



# revision 44
# speedup vs baseline: 1.2091x; 1.2091x over previous
"""Trainium2 Bass kernel for nn_Net_3152505995417 (gnn_message_passing) — v13.

Closed-form pair-layout reformulation of the two node convs + edge conv:
with T the incidence matrix of the complete graph, everything lives in a
dense [N, N] node-pair layout (slots (i,j)/(j,i) carry edge {i,j}).

Key structure (vs the v1 baseline at ~35.6 us):
  * bf16 end-to-end: 1-pass PE matmuls, halved DMA bytes
  * the k->m edge-feature mix G_m = sum_k relu(ea_k) We[k,m] runs on DVE
    right after the ea DMA lands; 1/colmax and -(dv_i+dv_j) fold in after
    as one per-plane multiply (q_m = G_m * crecN)
  * node_conv1 runs on the PE via PSUM accumulation:
    x1^T = sum_k (x @ p1_k W1)^T @ ea_k  (A1 = sum_k p1_k ea_k, ea sym.)
  * colmax reciprocal via per-node column: crec[i,j] = min(rec_i, rec_j),
    rec = 1/(max(dv,0)+eps) -- a [N,1] reciprocal instead of [N,N]
  * incidence sums S via one tt + one X-reduce; column side via PE
    transposes of St columns (symmetry: colsum == rowsum)
  * z = U + q accumulated on PE into two PSUM banks (planes 0-2 / 3-4);
    the final stage only needs row-sums: s = A2 @ 1, so e2/d2/A2/x2 are
    never materialized (accm_m = rowsum(relu(z_m + t1b_m)), minus the
    closed-form diagonal correction)
  * work split: DVE (mix, reductions, r), ACT (copies, relus, r0/r1),
    GpSimd (qfold, small tt), PE (all matmuls/broadcasts/transposes)
  * svec params ship pre-broadcast; ea ships row-split across two DMA
    queues with wide (1160B) partition lines
All 8 cores run the same replicated program (total work is a few hundred
KB; replication beats collective latency); core 0's output is returned.
"""

import numpy as np

N = 116
E = N * (N - 1) // 2
HID = 64
EDIM = 5
OUT = 4
ENC = HID + N // 2
EPS = 1e-10

# smalls column offsets
C_W1 = 0
C_W2 = 64
C_WL = 128
C_PET = 132
C_BENC = 133
C_B1 = 134
C_B2 = 135
C_BL = 136
C_SVEC = 140           # row 0: p1(5) p2(5) be(5) We k-major(25) We m-major(25)
SM_W = 208

_CACHE = {}


def _split_excess_waits(nc, mybir, max_waits=1):
    """This build accepts only one sync-wait per instruction; move excess
    waits onto chained NoOps on the same engine."""
    for fn in nc.m.functions:
        for blk in fn.blocks:
            insts = blk.instructions
            new, changed = [], False
            for ins in insts:
                si = ins.sync_info
                waits = list(si.on_wait) if si is not None else []
                if len(waits) > max_waits:
                    while len(waits) > max_waits:
                        chunk, waits = waits[:1], waits[1:]
                        nop = mybir.InstNoOp(
                            name=nc.get_next_instruction_name(),
                            engine=ins.engine,
                            sync_info=mybir.SyncInfo(on_wait=chunk, on_update=[]),
                            bass_nofuse=True,
                        )
                        new.append(nop)
                    si.on_wait = waits
                    changed = True
                new.append(ins)
            if changed:
                blk.instructions = new


def _build():
    import concourse.bass as bass
    import concourse.tile as tile
    from concourse import mybir
    from concourse.masks import make_identity

    f32 = mybir.dt.float32
    bf = mybir.dt.bfloat16
    A = mybir.AluOpType
    Relu = mybir.ActivationFunctionType.Relu
    Ident = mybir.ActivationFunctionType.Identity

    nc = bass.Bass("TRN2", target_bir_lowering=False, num_devices=8)

    eaT_d = nc.declare_dram_parameter("eaT", [64, EDIM * N], bf, isOutput=False)
    eaB_d = nc.declare_dram_parameter("eaB", [64, EDIM * N], bf, isOutput=False)
    smv_d = nc.declare_dram_parameter("smv", [128, SM_W + 72], bf, isOutput=False)
    encw_d = nc.declare_dram_parameter("encw", [128, N + HID], bf, isOutput=False)
    out_d = nc.declare_dram_parameter("out", [OUT, 1], f32, isOutput=True)

    with tile.TileContext(nc) as tc:
        with (
            tc.tile_pool(name="sb", bufs=1) as sb,
            tc.tile_pool(name="pm", bufs=3) as pm,
            tc.tile_pool(name="ps", bufs=3, space="PSUM") as ps,
            tc.tile_pool(name="psr", bufs=1, space="PSUM") as psr,
            tc.tile_pool(name="psu", bufs=2, space="PSUM") as psu,
        ):
            tEA = sb.tile([128, EDIM * N], bf, tag="tEA")
            tSV = sb.tile([128, SM_W + 72], bf, tag="tSV")
            tBC = sb.tile([128, N + HID], bf, tag="tBC")
            tSM = tSV[:, 0:SM_W]
            svecBh = tSV[0:N, SM_W:SM_W + 65]
            Wesq = tSV[0:EDIM, SM_W + 66:SM_W + 71]

            # ---- input DMAs: ea split by rows (wide lines, 2 queues) ----
            nc.sync.dma_start(out=tEA[0:64, :], in_=eaT_d[:])
            nc.gpsimd.dma_start(out=tEA[64:128, :], in_=eaB_d[:])
            nc.scalar.dma_start(out=tSV[:], in_=smv_d[:])
            nc.gpsimd.dma_start(out=tBC[:], in_=encw_d[:])

            encT = tBC[0:ENC, 0:N]
            Wenc = tBC[0:ENC, N:N + HID]
            W1 = tSM[0:HID, C_W1:C_W1 + HID]
            W2 = tSM[0:HID, C_W2:C_W2 + HID]
            WlA = tSM[0:HID + 1, C_WL:C_WL + OUT]      # row 64 = bl
            peT = tSM[0:HID, C_PET:C_PET + 1]

            # ---- constants (run while DMAs are in flight) ----
            ones_row = sb.tile([1, N], bf, tag="ones_row")
            nc.vector.memset(ones_row[:], 1.0)
            ident = sb.tile([N, N], bf, tag="ident")
            make_identity(nc, ident[:])
            warm = sb.tile([1, 2], bf, tag="warm")
            nc.scalar.activation(warm[:], ones_row[:, 0:2], Relu)
            zerosT = sb.tile([N, N], bf, tag="zerosT")
            nc.vector.memset(zerosT[:], 0.0)
            poolT = sb.tile([HID + 1, 1], bf, tag="poolT")
            nc.vector.memset(poolT[HID:HID + 1, 0:1], 1.0)

            # ---- svec arrives pre-broadcast in the slab ----
            bias32 = sb.tile([HID, 4], f32, tag="bias32")
            nc.scalar.activation(bias32[:], tSM[0:HID, C_BENC:C_BENC + 4], Ident)
            p2B = svecBh[:, 5:10]
            beB = svecBh[:, 10:15]

            # ---- eR = relu(ea)  (DVE) ----
            eR = sb.tile([N, EDIM * N], bf, tag="eR")
            nc.vector.tensor_scalar_max(eR[:], tEA[0:N, :], 0.0)
            # W1s[:, k-block] = p1_k * W1 (one early DVE op), then one wide
            # matmul gives all five xW1s blocks; PE accumulates
            # x1T = sum_k (x @ p1_k W1)^T @ ea_k   (A1 = sum p1_k ea_k)
            W1s = sb.tile([HID, EDIM * HID], bf, tag="W1s")
            nc.vector.tensor_tensor(
                W1s[:].rearrange("p (k h) -> p k h", k=EDIM),
                W1[:, None, :].to_broadcast([HID, EDIM, HID]),
                svecBh[0:HID, 0:5][:, :, None].to_broadcast([HID, EDIM, HID]),
                A.mult,
            )

            def eRk_b(k):
                return eR[:, k * N:(k + 1) * N][:, None, :].to_broadcast(
                    [N, EDIM, N]
                )

            def WeB_b(k):
                return svecBh[:, 15 + 5 * k:20 + 5 * k][:, :, None].to_broadcast(
                    [N, EDIM, N]
                )

            # ---- mix: G3[i,m,j] = sum_k eR_k[i,j] We[k,m] ----
            # mults: k=0,1,2 DVE / k=3,4 GpSimd; add tree: D:G+=Ta, G:Tb+=Tc,
            # D:G+=Td, D:G+=Tb
            G = sb.tile([N, EDIM * N], bf, tag="G")
            G3 = G[:].rearrange("p (m j) -> p m j", m=EDIM)
            Ta = sb.tile([N, EDIM * N], bf, tag="Ta")
            Ta3 = Ta[:].rearrange("p (m j) -> p m j", m=EDIM)
            Tb = sb.tile([N, EDIM * N], bf, tag="Tb")
            Tb3 = Tb[:].rearrange("p (m j) -> p m j", m=EDIM)
            Tc = sb.tile([N, EDIM * N], bf, tag="Tc")
            Tc3 = Tc[:].rearrange("p (m j) -> p m j", m=EDIM)
            Td = sb.tile([N, EDIM * N], bf, tag="Td")
            Td3 = Td[:].rearrange("p (m j) -> p m j", m=EDIM)

            nc.vector.tensor_tensor(G3, eRk_b(0), WeB_b(0), A.mult)
            nc.vector.tensor_tensor(Ta3, eRk_b(1), WeB_b(1), A.mult)
            nc.vector.tensor_tensor(Td3, eRk_b(2), WeB_b(2), A.mult)
            nc.vector.tensor_tensor(Tb3, eRk_b(3), WeB_b(3), A.mult)
            nc.vector.tensor_tensor(Tc3, eRk_b(4), WeB_b(4), A.mult)
            nc.vector.tensor_tensor(G3, G3, Ta3, A.add)
            nc.vector.tensor_tensor(G3, G3, Td3, A.add)

            # ---- x side (PE + ACT), overlapped with the mix ----
            xT_ps = ps.tile([HID, N], f32, tag="ps")
            nc.tensor.matmul(xT_ps[:], Wenc, encT, start=True, stop=True)
            xT = sb.tile([HID, N], bf, tag="xT")
            nc.scalar.activation(xT[:], xT_ps[:], Ident, bias=bias32[:, 0:1])
            xW1s_ps = ps.tile([N, EDIM * HID], f32, tag="ps")
            nc.tensor.matmul(xW1s_ps[:], xT[:], W1s[:], start=True, stop=True)
            xW1s = sb.tile([N, EDIM * HID], bf, tag="xW1s")
            nc.scalar.activation(xW1s[:], xW1s_ps[:], Ident)
            x1T_ps = ps.tile([HID, N], f32, tag="ps")
            for k in range(EDIM):
                nc.tensor.matmul(
                    x1T_ps[:], xW1s[:, k * HID:(k + 1) * HID],
                    tEA[0:N, k * N:(k + 1) * N],
                    start=(k == 0), stop=(k == 4),
                )
            x1T = sb.tile([HID, N], bf, tag="x1T")
            nc.scalar.activation(x1T[:], x1T_ps[:], Relu, bias=bias32[:, 1:2])

            dvr_ps = ps.tile([1, N], f32, tag="ps")
            nc.tensor.matmul(dvr_ps[:], peT, x1T[:], start=True, stop=True)
            dvT_ps = ps.tile([N, 1], f32, tag="ps")
            nc.tensor.matmul(dvT_ps[:], x1T[:], peT, start=True, stop=True)
            # dv copies + negsum on ACT
            dv_row = sb.tile([1, N], bf, tag="dv_row")
            nc.scalar.activation(dv_row[:], dvr_ps[:], Ident)
            dvTe = sb.tile([N, 1], f32, tag="dvTe")
            nc.scalar.activation(dvTe[:], dvT_ps[:], Ident)
            dvTn = sb.tile([N, 1], f32, tag="dvTn")
            nc.scalar.activation(dvTn[:], dvT_ps[:], Ident, scale=-1.0)
            dvROW_ps = psr.tile([N, N], f32, tag="psrow")
            nc.tensor.matmul(dvROW_ps[:], ones_row[:], dv_row[:], start=True, stop=True)
            negsum = sb.tile([N, N], bf, tag="negsum")
            nc.scalar.activation(
                negsum[:], dvROW_ps[:], Ident, scale=-1.0, bias=dvTn[:, 0:1]
            )
            # xW2 early (needed only at the very end)
            xW2_ps = ps.tile([N, HID], f32, tag="ps")
            nc.tensor.matmul(xW2_ps[:], x1T[:], W2, start=True, stop=True)
            xW2 = sb.tile([N, HID], bf, tag="xW2")
            nc.scalar.activation(xW2[:], xW2_ps[:], Ident)

            # ---- colmax reciprocal: 1/(max(dv,0)+eps) per node, then
            #      crec[i,j] = min(rec_i, rec_j) (f monotone decreasing) ----
            dvm = sb.tile([N, 1], f32, tag="dvm")
            nc.vector.tensor_scalar(dvm[:], dvT_ps[:], 0.0, EPS, A.max, A.add)
            rec32 = sb.tile([N, 1], f32, tag="rec32")
            nc.vector.reciprocal(rec32[:], dvm[:])
            recb = sb.tile([N, 1], bf, tag="recb")
            nc.vector.tensor_copy(recb[:], rec32[:])
            recT_ps = ps.tile([1, N], bf, tag="ps")
            nc.tensor.transpose(recT_ps[:], recb[:], ident[:])
            rec_row = sb.tile([1, N], bf, tag="rec_row")
            nc.vector.tensor_copy(rec_row[:], recT_ps[:])
            recROW_ps = ps.tile([N, N], f32, tag="ps")
            nc.tensor.matmul(
                recROW_ps[:], ones_row[:], rec_row[:], start=True, stop=True
            )
            crecb = sb.tile([N, N], bf, tag="crecb")
            nc.vector.tensor_scalar_min(crecb[:], recROW_ps[:], rec32[:, 0:1])
            crecN = sb.tile([N, N], bf, tag="crecN")
            nc.vector.tensor_tensor(crecN[:], crecb[:], negsum[:], A.mult)

            # ---- finish the mix (DVE), then qfold on GpSimd ----
            nc.vector.tensor_tensor(G3, G3, Tb3, A.add)
            nc.vector.tensor_tensor(G3, G3, Tc3, A.add)
            Q = sb.tile([N, EDIM * N], bf, tag="Q")
            for m in range(3):
                nc.gpsimd.tensor_tensor(
                    Q[:, m * N:(m + 1) * N], G[:, m * N:(m + 1) * N], crecN[:],
                    A.mult,
                )

            # ---- EC[i,k] = sum_j eR_k[i,j] crec[i,j]  (tt + X-reduce) ----
            ejk = sb.tile([N, EDIM * N], bf, tag="ejk")
            ejk3 = ejk[:].rearrange("p (k j) -> p k j", k=EDIM)
            nc.vector.tensor_tensor(
                ejk3, eR[:].rearrange("p (k j) -> p k j", k=EDIM),
                crecb[:, None, :].to_broadcast([N, EDIM, N]), A.mult,
            )
            EC = sb.tile([N, EDIM], bf, tag="EC")
            with nc.allow_low_precision(reason="bf16 net, 2e-2 tolerance"):
                nc.vector.tensor_reduce(
                    EC[:], ejk3, mybir.AxisListType.X, A.add
                )
            # ECd[i,k] = dv_i EC[i,k];  St[i,m] = sum_k ECd[i,k] We[k,m]
            ECd = sb.tile([N, EDIM], bf, tag="ECd")
            nc.vector.tensor_scalar_mul(ECd[:], EC[:], dvTe[:, 0:1])
            tmp3 = sb.tile([N, EDIM * EDIM], bf, tag="tmp3")
            nc.vector.tensor_tensor(
                tmp3[:].rearrange("p (m k) -> p m k", m=EDIM),
                ECd[:, None, :].to_broadcast([N, EDIM, EDIM]),
                svecBh[:, 40:65].rearrange("p (m k) -> p m k", m=EDIM),
                A.mult,
            )
            St = sb.tile([N, EDIM], bf, tag="St")
            with nc.allow_low_precision(reason="5-term sum, bf16 net"):
                nc.vector.tensor_reduce(
                    St[:], tmp3[:].rearrange("p (m k) -> p m k", m=EDIM),
                    mybir.AxisListType.X, A.add,
                )
            t1b = sb.tile([N, EDIM], f32, tag="t1b")
            nc.gpsimd.tensor_tensor(t1b[:], St[:], beB, A.add)
            for m in (3, 4):
                nc.vector.tensor_tensor(
                    Q[:, m * N:(m + 1) * N], G[:, m * N:(m + 1) * N], crecN[:],
                    A.mult,
                )

            # ---- u[(m,j)] = St[j,m] via per-column PE transposes ----
            u_psA = psr.tile([1, 3 * N], bf, tag="upsA")
            for m in range(3):
                nc.tensor.transpose(
                    u_psA[0:1, m * N:(m + 1) * N], St[:, m:m + 1], ident[:]
                )
            u_psB = psr.tile([1, 2 * N], bf, tag="upsB")
            for m in range(2):
                nc.tensor.transpose(
                    u_psB[0:1, m * N:(m + 1) * N], St[:, 3 + m:4 + m], ident[:]
                )
            u_sb = sb.tile([1, EDIM * N], bf, tag="u_sb")
            nc.scalar.activation(u_sb[0:1, 0:3 * N], u_psA[:], Ident)
            nc.vector.tensor_copy(u_sb[0:1, 3 * N:5 * N], u_psB[:])

            # ---- z = U + q accumulated on PE in two PSUM banks ----
            # bank A: planes 0-2, bank B: planes 3-4
            UA = psu.tile([N, 3 * N], f32, tag="psu")
            UB = psu.tile([N, 2 * N], f32, tag="psu")
            nc.tensor.matmul(UA[:], ident[:], Q[:, 0:3 * N],
                             start=True, stop=False)
            nc.tensor.matmul(UB[:], ident[:], Q[:, 3 * N:5 * N],
                             start=True, stop=False)
            nc.tensor.matmul(UA[:], ones_row[:], u_sb[0:1, 0:3 * N],
                             start=False, stop=True)
            nc.tensor.matmul(UB[:], ones_row[:], u_sb[0:1, 3 * N:5 * N],
                             start=False, stop=True)

            def z_m(m):
                if m < 3:
                    return UA[:, m * N:(m + 1) * N]
                return UB[:, (m - 3) * N:(m - 2) * N]

            # ---- accm_m = rowsum(relu(z_m + t1b_m)): 2 ACT + 3 DVE ----
            accmA = sb.tile([N, 2], f32, tag="accmA")
            accmD = sb.tile([N, 3], f32, tag="accmD")
            for m in range(EDIM):
                R = pm.tile([N, N], bf, tag="R")
                if m < 2:
                    nc.scalar.activation(
                        R[:], z_m(m), Relu, bias=t1b[:, m:m + 1],
                        accum_out=accmA[:, m:m + 1],
                    )
                else:
                    nc.vector.scalar_tensor_tensor(
                        R[:], z_m(m), t1b[:, m:m + 1], zerosT[:], A.add, A.max,
                        accum_out=accmD[:, m - 2:m - 1],
                    )

            # ---- diagonal correction (GpSimd + DVE, tiny) ----
            darg = sb.tile([N, EDIM], bf, tag="darg")
            nc.gpsimd.tensor_tensor(darg[:], St[:], t1b[:], A.add)
            dr = sb.tile([N, EDIM], bf, tag="dr")
            nc.vector.tensor_scalar_max(dr[:], darg[:], 0.0)
            ds = sb.tile([N, 1], f32, tag="ds")
            dscr = sb.tile([N, EDIM], bf, tag="dscr")
            nc.vector.scalar_tensor_tensor(
                dscr[:], dr[:], 1.0, p2B, A.mult, A.mult, accum_out=ds[:]
            )

            # ---- s = ((sum_m p2_m accm_m) - diag)/N; pooled head ----
            sA = sb.tile([N, 1], f32, tag="sA")
            sAscr = sb.tile([N, 2], bf, tag="sAscr")
            nc.vector.scalar_tensor_tensor(
                sAscr[:], accmA[:], 1.0, p2B[:, 0:2], A.mult, A.mult,
                accum_out=sA[:],
            )
            sD = sb.tile([N, 1], f32, tag="sD")
            sDscr = sb.tile([N, 3], bf, tag="sDscr")
            nc.vector.scalar_tensor_tensor(
                sDscr[:], accmD[:], 1.0, p2B[:, 2:5], A.mult, A.mult,
                accum_out=sD[:],
            )
            s0 = sb.tile([N, 1], f32, tag="s0")
            nc.vector.tensor_tensor(s0[:], sA[:], sD[:], A.add)
            sfin = sb.tile([N, 1], bf, tag="sfin")
            nc.vector.tensor_scalar(
                sfin[:], s0[:], ds[:, 0:1], 1.0 / N, A.subtract, A.mult
            )
            pooled_ps = ps.tile([HID, 1], f32, tag="ps")
            nc.tensor.matmul(pooled_ps[:], xW2[:], sfin[:], start=True, stop=True)
            nc.vector.tensor_scalar_add(
                poolT[0:HID, 0:1], pooled_ps[:], bias32[:, 2:3]
            )
            outT_ps = ps.tile([OUT, 1], f32, tag="ps")
            nc.tensor.matmul(outT_ps[:], WlA, poolT[:], start=True, stop=True)
            out_sb = sb.tile([OUT, 1], f32, tag="out_sb")
            nc.vector.tensor_copy(out_sb[:], outT_ps[:])
            nc.sync.dma_start(out=out_d[:], in_=out_sb[:])

    from concourse.library_overlay import lower_extended_insts

    lower_extended_insts(nc)
    _split_excess_waits(nc, mybir)
    return nc


def _prep_inputs(inputs):
    from ml_dtypes import bfloat16

    ei = np.asarray(inputs["edge_index"][0], dtype=np.int64)
    ej = np.asarray(inputs["edge_index"][1], dtype=np.int64)
    ea = np.asarray(inputs["edge_attr"], dtype=np.float32)

    ead = np.zeros((N, EDIM, N), dtype=np.float32)
    ead[ei, :, ej] = ea
    ead[ej, :, ei] = ea
    ead = ead.reshape(N, EDIM * N)

    smalls = np.zeros((128, SM_W), dtype=np.float32)
    smalls[0:HID, C_W1:C_W1 + HID] = np.asarray(inputs["W1"], np.float32)
    smalls[0:HID, C_W2:C_W2 + HID] = np.asarray(inputs["W2"], np.float32)
    smalls[0:HID, C_WL:C_WL + OUT] = np.asarray(inputs["Wl"], np.float32)
    smalls[HID, C_WL:C_WL + OUT] = np.asarray(inputs["bl"], np.float32).reshape(-1)
    smalls[0:HID, C_PET] = np.asarray(inputs["pe"], np.float32).reshape(-1)
    smalls[0:HID, C_BENC] = np.asarray(inputs["b_enc"], np.float32).reshape(-1)
    smalls[0:HID, C_B1] = np.asarray(inputs["b1"], np.float32).reshape(-1)
    smalls[0:HID, C_B2] = np.asarray(inputs["b2"], np.float32).reshape(-1)
    smalls[0:OUT, C_BL] = np.asarray(inputs["bl"], np.float32).reshape(-1)
    We = np.asarray(inputs["We"], np.float32)
    svec = np.concatenate(
        [
            np.asarray(inputs["p1"], np.float32).reshape(-1),
            np.asarray(inputs["p2"], np.float32).reshape(-1),
            np.asarray(inputs["be"], np.float32).reshape(-1),
            We.reshape(-1),          # k-major: col 15+5k+m
            We.T.reshape(-1),        # m-major: col 40+5m+k
        ]
    )
    # svec shipped pre-broadcast in smv instead

    encw = np.zeros((ENC, N + HID), dtype=np.float32)
    encw[:, 0:N] = np.asarray(inputs["encoding_raw"], np.float32).T
    encw[:, N:N + HID] = np.asarray(inputs["W_enc"], np.float32)

    def pad_rows(a):
        out = np.zeros((128, a.shape[1]), dtype=np.float32)
        out[0:a.shape[0]] = a
        return out

    def pad128(a):
        out = np.zeros((128, a.shape[1]), dtype=np.float32)
        out[0:a.shape[0]] = a
        return out.astype(bfloat16)

    svb = np.zeros((128, 72), dtype=np.float32)
    svb[0:N, 0:65] = svec[None, :]
    svb[0:EDIM, 66:71] = We
    smv = np.concatenate([smalls[0:128], svb], axis=1)
    eadp = np.zeros((128, EDIM * N), dtype=np.float32)
    eadp[0:N] = ead
    return {
        "eaT": eadp[0:64].astype(bfloat16),
        "eaB": eadp[64:128].astype(bfloat16),
        "smv": smv.astype(bfloat16),
        "encw": pad_rows(encw).astype(bfloat16),
    }


def kernel(**inputs) -> np.ndarray:
    import sys

    if "/opt/trn_rl_repo" not in sys.path:
        sys.path.insert(0, "/opt/trn_rl_repo")
    from concourse.bass_utils import run_bass_kernel_spmd

    if "nc" not in _CACHE:
        _CACHE["nc"] = _build()
    nc = _CACHE["nc"]

    in_map = _prep_inputs(inputs)
    res = run_bass_kernel_spmd(
        nc, [in_map] * 8, core_ids=list(range(8)), trace=False
    )
    return np.asarray(res.results[0]["out"], dtype=np.float32).reshape(1, OUT)


# revision 45
# speedup vs baseline: 1.2384x; 1.0243x over previous
"""Trainium2 Bass kernel for nn_Net_3152505995417 (gnn_message_passing) — v13.

Closed-form pair-layout reformulation of the two node convs + edge conv:
with T the incidence matrix of the complete graph, everything lives in a
dense [N, N] node-pair layout (slots (i,j)/(j,i) carry edge {i,j}).

Key structure (vs the v1 baseline at ~35.6 us):
  * bf16 end-to-end: 1-pass PE matmuls, halved DMA bytes
  * the k->m edge-feature mix G_m = sum_k relu(ea_k) We[k,m] runs on DVE
    right after the ea DMA lands; 1/colmax and -(dv_i+dv_j) fold in after
    as one per-plane multiply (q_m = G_m * crecN)
  * node_conv1 runs on the PE via PSUM accumulation:
    x1^T = sum_k (x @ p1_k W1)^T @ ea_k  (A1 = sum_k p1_k ea_k, ea sym.)
  * colmax reciprocal via per-node column: crec[i,j] = min(rec_i, rec_j),
    rec = 1/(max(dv,0)+eps) -- a [N,1] reciprocal instead of [N,N]
  * incidence sums S via one tt + one X-reduce; column side via PE
    transposes of St columns (symmetry: colsum == rowsum)
  * z = U + q accumulated on PE into two PSUM banks (planes 0-2 / 3-4);
    the final stage only needs row-sums: s = A2 @ 1, so e2/d2/A2/x2 are
    never materialized (accm_m = rowsum(relu(z_m + t1b_m)), minus the
    closed-form diagonal correction)
  * work split: DVE (mix, reductions, r), ACT (copies, relus, r0/r1),
    GpSimd (qfold, small tt), PE (all matmuls/broadcasts/transposes)
  * svec params ship pre-broadcast; ea ships row-split across two DMA
    queues with wide (1160B) partition lines
All 8 cores run the same replicated program (total work is a few hundred
KB; replication beats collective latency); core 0's output is returned.
"""

import numpy as np

N = 116
E = N * (N - 1) // 2
HID = 64
EDIM = 5
OUT = 4
ENC = HID + N // 2
EPS = 1e-10

# smalls column offsets
C_W1 = 0
C_W2 = 64
C_WL = 128
C_PET = 132
C_BENC = 133
C_B1 = 134
C_B2 = 135
C_BL = 136
C_SVEC = 140           # row 0: p1(5) p2(5) be(5) We k-major(25) We m-major(25)
SM_W = 208

_CACHE = {}


def _split_excess_waits(nc, mybir, max_waits=1):
    """This build accepts only one sync-wait per instruction; move excess
    waits onto chained NoOps on the same engine."""
    for fn in nc.m.functions:
        for blk in fn.blocks:
            insts = blk.instructions
            new, changed = [], False
            for ins in insts:
                si = ins.sync_info
                waits = list(si.on_wait) if si is not None else []
                if len(waits) > max_waits:
                    while len(waits) > max_waits:
                        chunk, waits = waits[:1], waits[1:]
                        nop = mybir.InstNoOp(
                            name=nc.get_next_instruction_name(),
                            engine=ins.engine,
                            sync_info=mybir.SyncInfo(on_wait=chunk, on_update=[]),
                            bass_nofuse=True,
                        )
                        new.append(nop)
                    si.on_wait = waits
                    changed = True
                new.append(ins)
            if changed:
                blk.instructions = new


def _build():
    import concourse.bass as bass
    import concourse.tile as tile
    from concourse import mybir
    from concourse.masks import make_identity

    f32 = mybir.dt.float32
    bf = mybir.dt.bfloat16
    A = mybir.AluOpType
    Relu = mybir.ActivationFunctionType.Relu
    Ident = mybir.ActivationFunctionType.Identity

    nc = bass.Bass("TRN2", target_bir_lowering=False, num_devices=8)

    eaT_d = nc.declare_dram_parameter("eaT", [64, EDIM * N], bf, isOutput=False)
    eaB_d = nc.declare_dram_parameter("eaB", [64, EDIM * N], bf, isOutput=False)
    smv_d = nc.declare_dram_parameter("smv", [128, SM_W + 72], bf, isOutput=False)
    encw_d = nc.declare_dram_parameter("encw", [128, N + HID], bf, isOutput=False)
    out_d = nc.declare_dram_parameter("out", [OUT, 1], f32, isOutput=True)

    with tile.TileContext(nc) as tc:
        with (
            tc.tile_pool(name="sb", bufs=1) as sb,
            tc.tile_pool(name="pm", bufs=3) as pm,
            tc.tile_pool(name="ps", bufs=3, space="PSUM") as ps,
            tc.tile_pool(name="psr", bufs=1, space="PSUM") as psr,
            tc.tile_pool(name="psu", bufs=2, space="PSUM") as psu,
        ):
            tEA = sb.tile([128, EDIM * N], bf, tag="tEA")
            tSV = sb.tile([128, SM_W + 72], bf, tag="tSV")
            tBC = sb.tile([128, N + HID], bf, tag="tBC")
            tSM = tSV[:, 0:SM_W]
            svecBh = tSV[0:N, SM_W:SM_W + 65]
            Wesq = tSV[0:EDIM, SM_W + 66:SM_W + 71]

            # ---- input DMAs: ea split by rows (wide lines, 2 queues) ----
            nc.sync.dma_start(out=tEA[0:64, :], in_=eaT_d[:])
            nc.gpsimd.dma_start(out=tEA[64:128, :], in_=eaB_d[:])
            nc.scalar.dma_start(out=tSV[:], in_=smv_d[:])
            nc.gpsimd.dma_start(out=tBC[:], in_=encw_d[:])

            encT = tBC[0:ENC, 0:N]
            Wenc = tBC[0:ENC, N:N + HID]
            W1 = tSM[0:HID, C_W1:C_W1 + HID]
            W2 = tSM[0:HID, C_W2:C_W2 + HID]
            WlA = tSM[0:HID + 1, C_WL:C_WL + OUT]      # row 64 = bl
            peT = tSM[0:HID, C_PET:C_PET + 1]

            # ---- constants (run while DMAs are in flight) ----
            ones_row = sb.tile([1, N], bf, tag="ones_row")
            nc.vector.memset(ones_row[:], 1.0)
            ident = sb.tile([N, N], bf, tag="ident")
            make_identity(nc, ident[:])
            warm = sb.tile([1, 2], bf, tag="warm")
            nc.scalar.activation(warm[:], ones_row[:, 0:2], Relu)
            zerosT = sb.tile([N, N], bf, tag="zerosT")
            nc.vector.memset(zerosT[:], 0.0)
            poolT = sb.tile([HID + 1, 1], bf, tag="poolT")
            nc.vector.memset(poolT[HID:HID + 1, 0:1], 1.0)

            # ---- svec arrives pre-broadcast in the slab ----
            bias32 = sb.tile([HID, 4], f32, tag="bias32")
            nc.scalar.activation(bias32[:], tSM[0:HID, C_BENC:C_BENC + 4], Ident)
            p2B = svecBh[:, 5:10]
            beB = svecBh[:, 10:15]

            # ---- eR = relu(ea)  (DVE) ----
            eR = sb.tile([N, EDIM * N], bf, tag="eR")
            nc.vector.tensor_scalar_max(eR[:], tEA[0:N, :], 0.0)
            # W1s[:, k-block] = p1_k * W1 (one early DVE op), then one wide
            # matmul gives all five xW1s blocks; PE accumulates
            # x1T = sum_k (x @ p1_k W1)^T @ ea_k   (A1 = sum p1_k ea_k)
            W1s = sb.tile([HID, EDIM * HID], bf, tag="W1s")
            nc.vector.tensor_tensor(
                W1s[:].rearrange("p (k h) -> p k h", k=EDIM),
                W1[:, None, :].to_broadcast([HID, EDIM, HID]),
                svecBh[0:HID, 0:5][:, :, None].to_broadcast([HID, EDIM, HID]),
                A.mult,
            )

            def eRk_b(k):
                return eR[:, k * N:(k + 1) * N][:, None, :].to_broadcast(
                    [N, EDIM, N]
                )

            def WeB_b(k):
                return svecBh[:, 15 + 5 * k:20 + 5 * k][:, :, None].to_broadcast(
                    [N, EDIM, N]
                )

            # ---- mix: G3[i,m,j] = sum_k eR_k[i,j] We[k,m] ----
            # mults: k=0,1,2 DVE / k=3,4 GpSimd; add tree: D:G+=Ta, G:Tb+=Tc,
            # D:G+=Td, D:G+=Tb
            G = sb.tile([N, EDIM * N], bf, tag="G")
            G3 = G[:].rearrange("p (m j) -> p m j", m=EDIM)
            Ta = sb.tile([N, EDIM * N], bf, tag="Ta")
            Ta3 = Ta[:].rearrange("p (m j) -> p m j", m=EDIM)
            Tb = sb.tile([N, EDIM * N], bf, tag="Tb")
            Tb3 = Tb[:].rearrange("p (m j) -> p m j", m=EDIM)
            Tc = sb.tile([N, EDIM * N], bf, tag="Tc")
            Tc3 = Tc[:].rearrange("p (m j) -> p m j", m=EDIM)
            Td = sb.tile([N, EDIM * N], bf, tag="Td")
            Td3 = Td[:].rearrange("p (m j) -> p m j", m=EDIM)

            nc.vector.tensor_tensor(G3, eRk_b(0), WeB_b(0), A.mult)
            nc.vector.tensor_tensor(Ta3, eRk_b(1), WeB_b(1), A.mult)
            nc.vector.tensor_tensor(Td3, eRk_b(2), WeB_b(2), A.mult)
            nc.vector.tensor_tensor(Tb3, eRk_b(3), WeB_b(3), A.mult)
            nc.vector.tensor_tensor(Tc3, eRk_b(4), WeB_b(4), A.mult)
            nc.vector.tensor_tensor(G3, G3, Ta3, A.add)
            nc.vector.tensor_tensor(G3, G3, Td3, A.add)
            nc.gpsimd.tensor_tensor(Tb3, Tb3, Tc3, A.add)

            # ---- x side (PE + ACT), overlapped with the mix ----
            xT_ps = ps.tile([HID, N], f32, tag="ps")
            nc.tensor.matmul(xT_ps[:], Wenc, encT, start=True, stop=True)
            xT = sb.tile([HID, N], bf, tag="xT")
            nc.scalar.activation(xT[:], xT_ps[:], Ident, bias=bias32[:, 0:1])
            xW1s_ps = ps.tile([N, EDIM * HID], f32, tag="ps")
            nc.tensor.matmul(xW1s_ps[:], xT[:], W1s[:], start=True, stop=True)
            xW1s = sb.tile([N, EDIM * HID], bf, tag="xW1s")
            nc.scalar.activation(xW1s[:], xW1s_ps[:], Ident)
            x1T_ps = ps.tile([HID, N], f32, tag="ps")
            for k in range(EDIM):
                nc.tensor.matmul(
                    x1T_ps[:], xW1s[:, k * HID:(k + 1) * HID],
                    tEA[0:N, k * N:(k + 1) * N],
                    start=(k == 0), stop=(k == 4),
                )
            x1T = sb.tile([HID, N], bf, tag="x1T")
            nc.scalar.activation(x1T[:], x1T_ps[:], Relu, bias=bias32[:, 1:2])

            dvr_ps = ps.tile([1, N], f32, tag="ps")
            nc.tensor.matmul(dvr_ps[:], peT, x1T[:], start=True, stop=True)
            dvT_ps = ps.tile([N, 1], f32, tag="ps")
            nc.tensor.matmul(dvT_ps[:], x1T[:], peT, start=True, stop=True)
            # dv copies + negsum on ACT
            dv_row = sb.tile([1, N], bf, tag="dv_row")
            nc.scalar.activation(dv_row[:], dvr_ps[:], Ident)
            dvTe = sb.tile([N, 1], f32, tag="dvTe")
            nc.scalar.activation(dvTe[:], dvT_ps[:], Ident)
            dvTn = sb.tile([N, 1], f32, tag="dvTn")
            nc.scalar.activation(dvTn[:], dvT_ps[:], Ident, scale=-1.0)
            dvROW_ps = psr.tile([N, N], f32, tag="psrow")
            nc.tensor.matmul(dvROW_ps[:], ones_row[:], dv_row[:], start=True, stop=True)
            negsum = sb.tile([N, N], bf, tag="negsum")
            nc.scalar.activation(
                negsum[:], dvROW_ps[:], Ident, scale=-1.0, bias=dvTn[:, 0:1]
            )
            # xW2 early (needed only at the very end)
            xW2_ps = ps.tile([N, HID], f32, tag="ps")
            nc.tensor.matmul(xW2_ps[:], x1T[:], W2, start=True, stop=True)
            xW2 = sb.tile([N, HID], bf, tag="xW2")
            nc.scalar.activation(xW2[:], xW2_ps[:], Ident)

            # ---- colmax reciprocal: 1/(max(dv,0)+eps) per node, then
            #      crec[i,j] = min(rec_i, rec_j) (f monotone decreasing) ----
            dvm = sb.tile([N, 1], f32, tag="dvm")
            nc.vector.tensor_scalar(dvm[:], dvT_ps[:], 0.0, EPS, A.max, A.add)
            rec32 = sb.tile([N, 1], f32, tag="rec32")
            nc.vector.reciprocal(rec32[:], dvm[:])
            recb = sb.tile([N, 1], bf, tag="recb")
            nc.vector.tensor_copy(recb[:], rec32[:])
            recT_ps = ps.tile([1, N], bf, tag="ps")
            nc.tensor.transpose(recT_ps[:], recb[:], ident[:])
            rec_row = sb.tile([1, N], bf, tag="rec_row")
            nc.vector.tensor_copy(rec_row[:], recT_ps[:])
            recROW_ps = ps.tile([N, N], f32, tag="ps")
            nc.tensor.matmul(
                recROW_ps[:], ones_row[:], rec_row[:], start=True, stop=True
            )
            crecb = sb.tile([N, N], bf, tag="crecb")
            nc.vector.tensor_scalar_min(crecb[:], recROW_ps[:], rec32[:, 0:1])
            crecN = sb.tile([N, N], bf, tag="crecN")
            nc.vector.tensor_tensor(crecN[:], crecb[:], negsum[:], A.mult)

            # ---- finish the mix: single DVE add (Tb+Tc folded on GpSimd) ----
            nc.vector.tensor_tensor(G3, G3, Tb3, A.add)
            Q = sb.tile([N, EDIM * N], bf, tag="Q")
            for m in range(3):
                nc.gpsimd.tensor_tensor(
                    Q[:, m * N:(m + 1) * N], G[:, m * N:(m + 1) * N], crecN[:],
                    A.mult,
                )

            # ---- EC[i,k] = sum_j eR_k[i,j] crec[i,j]  (tt + X-reduce) ----
            ejk = sb.tile([N, EDIM * N], bf, tag="ejk")
            ejk3 = ejk[:].rearrange("p (k j) -> p k j", k=EDIM)
            nc.vector.tensor_tensor(
                ejk3, eR[:].rearrange("p (k j) -> p k j", k=EDIM),
                crecb[:, None, :].to_broadcast([N, EDIM, N]), A.mult,
            )
            EC = sb.tile([N, EDIM], bf, tag="EC")
            with nc.allow_low_precision(reason="bf16 net, 2e-2 tolerance"):
                nc.vector.tensor_reduce(
                    EC[:], ejk3, mybir.AxisListType.X, A.add
                )
            # ECd[i,k] = dv_i EC[i,k];  St[i,m] = sum_k ECd[i,k] We[k,m]
            ECd = sb.tile([N, EDIM], bf, tag="ECd")
            nc.vector.tensor_scalar_mul(ECd[:], EC[:], dvTe[:, 0:1])
            tmp3 = sb.tile([N, EDIM * EDIM], bf, tag="tmp3")
            nc.vector.tensor_tensor(
                tmp3[:].rearrange("p (m k) -> p m k", m=EDIM),
                ECd[:, None, :].to_broadcast([N, EDIM, EDIM]),
                svecBh[:, 40:65].rearrange("p (m k) -> p m k", m=EDIM),
                A.mult,
            )
            St = sb.tile([N, EDIM], bf, tag="St")
            with nc.allow_low_precision(reason="5-term sum, bf16 net"):
                nc.vector.tensor_reduce(
                    St[:], tmp3[:].rearrange("p (m k) -> p m k", m=EDIM),
                    mybir.AxisListType.X, A.add,
                )
            t1b = sb.tile([N, EDIM], f32, tag="t1b")
            nc.gpsimd.tensor_tensor(t1b[:], St[:], beB, A.add)
            for m in (3, 4):
                nc.vector.tensor_tensor(
                    Q[:, m * N:(m + 1) * N], G[:, m * N:(m + 1) * N], crecN[:],
                    A.mult,
                )

            # ---- u[(m,j)] = St[j,m] via per-column PE transposes ----
            u_psA = psr.tile([1, 3 * N], bf, tag="upsA")
            for m in range(3):
                nc.tensor.transpose(
                    u_psA[0:1, m * N:(m + 1) * N], St[:, m:m + 1], ident[:]
                )
            u_psB = psr.tile([1, 2 * N], bf, tag="upsB")
            for m in range(2):
                nc.tensor.transpose(
                    u_psB[0:1, m * N:(m + 1) * N], St[:, 3 + m:4 + m], ident[:]
                )
            u_sb = sb.tile([1, EDIM * N], bf, tag="u_sb")
            nc.scalar.activation(u_sb[0:1, 0:3 * N], u_psA[:], Ident)
            nc.vector.tensor_copy(u_sb[0:1, 3 * N:5 * N], u_psB[:])

            # ---- z = U + q accumulated on PE in two PSUM banks ----
            # bank A: planes 0-2, bank B: planes 3-4
            UA = psu.tile([N, 3 * N], f32, tag="psu")
            UB = psu.tile([N, 2 * N], f32, tag="psu")
            nc.tensor.matmul(UA[:], ident[:], Q[:, 0:3 * N],
                             start=True, stop=False)
            nc.tensor.matmul(UB[:], ident[:], Q[:, 3 * N:5 * N],
                             start=True, stop=False)
            nc.tensor.matmul(UA[:], ones_row[:], u_sb[0:1, 0:3 * N],
                             start=False, stop=True)
            nc.tensor.matmul(UB[:], ones_row[:], u_sb[0:1, 3 * N:5 * N],
                             start=False, stop=True)

            def z_m(m):
                if m < 3:
                    return UA[:, m * N:(m + 1) * N]
                return UB[:, (m - 3) * N:(m - 2) * N]

            # ---- accm_m = rowsum(relu(z_m + t1b_m)): 2 ACT + 3 DVE ----
            accmA = sb.tile([N, 2], f32, tag="accmA")
            accmD = sb.tile([N, 3], f32, tag="accmD")
            for m in range(EDIM):
                R = pm.tile([N, N], bf, tag="R")
                if m < 2:
                    nc.scalar.activation(
                        R[:], z_m(m), Relu, bias=t1b[:, m:m + 1],
                        accum_out=accmA[:, m:m + 1],
                    )
                else:
                    nc.vector.scalar_tensor_tensor(
                        R[:], z_m(m), t1b[:, m:m + 1], zerosT[:], A.add, A.max,
                        accum_out=accmD[:, m - 2:m - 1],
                    )

            # ---- diagonal correction (GpSimd + DVE, tiny) ----
            darg = sb.tile([N, EDIM], bf, tag="darg")
            nc.gpsimd.tensor_tensor(darg[:], St[:], t1b[:], A.add)
            dr = sb.tile([N, EDIM], bf, tag="dr")
            nc.vector.tensor_scalar_max(dr[:], darg[:], 0.0)
            ds = sb.tile([N, 1], f32, tag="ds")
            dscr = sb.tile([N, EDIM], bf, tag="dscr")
            nc.vector.scalar_tensor_tensor(
                dscr[:], dr[:], 1.0, p2B, A.mult, A.mult, accum_out=ds[:]
            )

            # ---- s = ((sum_m p2_m accm_m) - diag)/N; pooled head ----
            sA = sb.tile([N, 1], f32, tag="sA")
            sAscr = sb.tile([N, 2], bf, tag="sAscr")
            nc.vector.scalar_tensor_tensor(
                sAscr[:], accmA[:], 1.0, p2B[:, 0:2], A.mult, A.mult,
                accum_out=sA[:],
            )
            sD = sb.tile([N, 1], f32, tag="sD")
            sDscr = sb.tile([N, 3], bf, tag="sDscr")
            nc.vector.scalar_tensor_tensor(
                sDscr[:], accmD[:], 1.0, p2B[:, 2:5], A.mult, A.mult,
                accum_out=sD[:],
            )
            s0 = sb.tile([N, 1], f32, tag="s0")
            nc.vector.tensor_tensor(s0[:], sA[:], sD[:], A.add)
            sfin = sb.tile([N, 1], bf, tag="sfin")
            nc.vector.tensor_scalar(
                sfin[:], s0[:], ds[:, 0:1], 1.0 / N, A.subtract, A.mult
            )
            pooled_ps = ps.tile([HID, 1], f32, tag="ps")
            nc.tensor.matmul(pooled_ps[:], xW2[:], sfin[:], start=True, stop=True)
            nc.vector.tensor_scalar_add(
                poolT[0:HID, 0:1], pooled_ps[:], bias32[:, 2:3]
            )
            outT_ps = ps.tile([OUT, 1], f32, tag="ps")
            nc.tensor.matmul(outT_ps[:], WlA, poolT[:], start=True, stop=True)
            out_sb = sb.tile([OUT, 1], f32, tag="out_sb")
            nc.vector.tensor_copy(out_sb[:], outT_ps[:])
            nc.sync.dma_start(out=out_d[:], in_=out_sb[:])

    from concourse.library_overlay import lower_extended_insts

    lower_extended_insts(nc)
    _split_excess_waits(nc, mybir)
    return nc


def _prep_inputs(inputs):
    from ml_dtypes import bfloat16

    ei = np.asarray(inputs["edge_index"][0], dtype=np.int64)
    ej = np.asarray(inputs["edge_index"][1], dtype=np.int64)
    ea = np.asarray(inputs["edge_attr"], dtype=np.float32)

    ead = np.zeros((N, EDIM, N), dtype=np.float32)
    ead[ei, :, ej] = ea
    ead[ej, :, ei] = ea
    ead = ead.reshape(N, EDIM * N)

    smalls = np.zeros((128, SM_W), dtype=np.float32)
    smalls[0:HID, C_W1:C_W1 + HID] = np.asarray(inputs["W1"], np.float32)
    smalls[0:HID, C_W2:C_W2 + HID] = np.asarray(inputs["W2"], np.float32)
    smalls[0:HID, C_WL:C_WL + OUT] = np.asarray(inputs["Wl"], np.float32)
    smalls[HID, C_WL:C_WL + OUT] = np.asarray(inputs["bl"], np.float32).reshape(-1)
    smalls[0:HID, C_PET] = np.asarray(inputs["pe"], np.float32).reshape(-1)
    smalls[0:HID, C_BENC] = np.asarray(inputs["b_enc"], np.float32).reshape(-1)
    smalls[0:HID, C_B1] = np.asarray(inputs["b1"], np.float32).reshape(-1)
    smalls[0:HID, C_B2] = np.asarray(inputs["b2"], np.float32).reshape(-1)
    smalls[0:OUT, C_BL] = np.asarray(inputs["bl"], np.float32).reshape(-1)
    We = np.asarray(inputs["We"], np.float32)
    svec = np.concatenate(
        [
            np.asarray(inputs["p1"], np.float32).reshape(-1),
            np.asarray(inputs["p2"], np.float32).reshape(-1),
            np.asarray(inputs["be"], np.float32).reshape(-1),
            We.reshape(-1),          # k-major: col 15+5k+m
            We.T.reshape(-1),        # m-major: col 40+5m+k
        ]
    )
    # svec shipped pre-broadcast in smv instead

    encw = np.zeros((ENC, N + HID), dtype=np.float32)
    encw[:, 0:N] = np.asarray(inputs["encoding_raw"], np.float32).T
    encw[:, N:N + HID] = np.asarray(inputs["W_enc"], np.float32)

    def pad_rows(a):
        out = np.zeros((128, a.shape[1]), dtype=np.float32)
        out[0:a.shape[0]] = a
        return out

    def pad128(a):
        out = np.zeros((128, a.shape[1]), dtype=np.float32)
        out[0:a.shape[0]] = a
        return out.astype(bfloat16)

    svb = np.zeros((128, 72), dtype=np.float32)
    svb[0:N, 0:65] = svec[None, :]
    svb[0:EDIM, 66:71] = We
    smv = np.concatenate([smalls[0:128], svb], axis=1)
    eadp = np.zeros((128, EDIM * N), dtype=np.float32)
    eadp[0:N] = ead
    return {
        "eaT": eadp[0:64].astype(bfloat16),
        "eaB": eadp[64:128].astype(bfloat16),
        "smv": smv.astype(bfloat16),
        "encw": pad_rows(encw).astype(bfloat16),
    }


def kernel(**inputs) -> np.ndarray:
    import sys

    if "/opt/trn_rl_repo" not in sys.path:
        sys.path.insert(0, "/opt/trn_rl_repo")
    from concourse.bass_utils import run_bass_kernel_spmd

    if "nc" not in _CACHE:
        _CACHE["nc"] = _build()
    nc = _CACHE["nc"]

    in_map = _prep_inputs(inputs)
    res = run_bass_kernel_spmd(
        nc, [in_map] * 8, core_ids=list(range(8)), trace=False
    )
    return np.asarray(res.results[0]["out"], dtype=np.float32).reshape(1, OUT)


# revision 46
# speedup vs baseline: 1.2949x; 1.0456x over previous
"""Trainium2 Bass kernel for nn_Net_3152505995417 (gnn_message_passing) — v13.

Closed-form pair-layout reformulation of the two node convs + edge conv:
with T the incidence matrix of the complete graph, everything lives in a
dense [N, N] node-pair layout (slots (i,j)/(j,i) carry edge {i,j}).

Key structure (vs the v1 baseline at ~35.6 us):
  * bf16 end-to-end: 1-pass PE matmuls, halved DMA bytes
  * the k->m edge-feature mix G_m = sum_k relu(ea_k) We[k,m] runs on DVE
    right after the ea DMA lands; 1/colmax and -(dv_i+dv_j) fold in after
    as one per-plane multiply (q_m = G_m * crecN)
  * node_conv1 runs on the PE via PSUM accumulation:
    x1^T = sum_k (x @ p1_k W1)^T @ ea_k  (A1 = sum_k p1_k ea_k, ea sym.)
  * colmax reciprocal via per-node column: crec[i,j] = min(rec_i, rec_j),
    rec = 1/(max(dv,0)+eps) -- a [N,1] reciprocal instead of [N,N]
  * incidence sums S via one tt + one X-reduce; column side via PE
    transposes of St columns (symmetry: colsum == rowsum)
  * z = U + q accumulated on PE into two PSUM banks (planes 0-2 / 3-4);
    the final stage only needs row-sums: s = A2 @ 1, so e2/d2/A2/x2 are
    never materialized (accm_m = rowsum(relu(z_m + t1b_m)), minus the
    closed-form diagonal correction)
  * work split: DVE (mix, reductions, r), ACT (copies, relus, r0/r1),
    GpSimd (qfold, small tt), PE (all matmuls/broadcasts/transposes)
  * svec params ship pre-broadcast; ea ships row-split across two DMA
    queues with wide (1160B) partition lines
All 8 cores run the same replicated program (total work is a few hundred
KB; replication beats collective latency); core 0's output is returned.
"""

import numpy as np

N = 116
E = N * (N - 1) // 2
HID = 64
EDIM = 5
OUT = 4
ENC = HID + N // 2
EPS = 1e-10

# smalls column offsets
C_W1 = 0
C_W2 = 64
C_WL = 128
C_PET = 132
C_BENC = 133
C_B1 = 134
C_B2 = 135
C_BL = 136
C_SVEC = 140           # row 0: p1(5) p2(5) be(5) We k-major(25) We m-major(25)
SM_W = 208

_CACHE = {}


def _split_excess_waits(nc, mybir, max_waits=1):
    """This build accepts only one sync-wait per instruction; move excess
    waits onto chained NoOps on the same engine."""
    for fn in nc.m.functions:
        for blk in fn.blocks:
            insts = blk.instructions
            new, changed = [], False
            for ins in insts:
                si = ins.sync_info
                waits = list(si.on_wait) if si is not None else []
                if len(waits) > max_waits:
                    while len(waits) > max_waits:
                        chunk, waits = waits[:1], waits[1:]
                        nop = mybir.InstNoOp(
                            name=nc.get_next_instruction_name(),
                            engine=ins.engine,
                            sync_info=mybir.SyncInfo(on_wait=chunk, on_update=[]),
                            bass_nofuse=True,
                        )
                        new.append(nop)
                    si.on_wait = waits
                    changed = True
                new.append(ins)
            if changed:
                blk.instructions = new


def _build():
    import concourse.bass as bass
    import concourse.tile as tile
    from concourse import mybir
    from concourse.masks import make_identity

    f32 = mybir.dt.float32
    bf = mybir.dt.bfloat16
    A = mybir.AluOpType
    Relu = mybir.ActivationFunctionType.Relu
    Ident = mybir.ActivationFunctionType.Identity

    nc = bass.Bass("TRN2", target_bir_lowering=False, num_devices=8)

    eaT_d = nc.declare_dram_parameter("eaT", [64, EDIM * N], bf, isOutput=False)
    eaB_d = nc.declare_dram_parameter("eaB", [64, EDIM * N], bf, isOutput=False)
    smv_d = nc.declare_dram_parameter("smv", [128, SM_W + 72], bf, isOutput=False)
    encw_d = nc.declare_dram_parameter("encw", [128, N + HID], bf, isOutput=False)
    out_d = nc.declare_dram_parameter("out", [OUT, 1], f32, isOutput=True)

    with tile.TileContext(nc) as tc:
        with (
            tc.tile_pool(name="sb", bufs=1) as sb,
            tc.tile_pool(name="pm", bufs=3) as pm,
            tc.tile_pool(name="ps", bufs=3, space="PSUM") as ps,
            tc.tile_pool(name="psr", bufs=1, space="PSUM") as psr,
            tc.tile_pool(name="psu", bufs=2, space="PSUM") as psu,
        ):
            tEA = sb.tile([128, EDIM * N], bf, tag="tEA")
            tSV = sb.tile([128, SM_W + 72], bf, tag="tSV")
            tBC = sb.tile([128, N + HID], bf, tag="tBC")
            tSM = tSV[:, 0:SM_W]
            svecBh = tSV[0:N, SM_W:SM_W + 65]
            Wesq = tSV[0:EDIM, SM_W + 66:SM_W + 71]

            # ---- input DMAs: ea split by rows (wide lines, 2 queues) ----
            nc.sync.dma_start(out=tEA[0:64, :], in_=eaT_d[:])
            nc.gpsimd.dma_start(out=tEA[64:128, :], in_=eaB_d[:])
            nc.scalar.dma_start(out=tSV[:], in_=smv_d[:])
            nc.gpsimd.dma_start(out=tBC[:], in_=encw_d[:])

            encT = tBC[0:ENC, 0:N]
            Wenc = tBC[0:ENC, N:N + HID]
            W1 = tSM[0:HID, C_W1:C_W1 + HID]
            W2 = tSM[0:HID, C_W2:C_W2 + HID]
            WlA = tSM[0:HID + 1, C_WL:C_WL + OUT]      # row 64 = bl
            peT = tSM[0:HID, C_PET:C_PET + 1]

            # ---- constants (run while DMAs are in flight) ----
            ones_row = sb.tile([1, N], bf, tag="ones_row")
            nc.vector.memset(ones_row[:], 1.0)
            ident = sb.tile([N, N], bf, tag="ident")
            make_identity(nc, ident[:])
            warm = sb.tile([1, 2], bf, tag="warm")
            nc.scalar.activation(warm[:], ones_row[:, 0:2], Relu)
            zerosT = sb.tile([N, N], bf, tag="zerosT")
            nc.vector.memset(zerosT[:], 0.0)
            poolT = sb.tile([HID + 1, 1], bf, tag="poolT")
            nc.vector.memset(poolT[HID:HID + 1, 0:1], 1.0)

            # ---- svec arrives pre-broadcast in the slab ----
            bias32 = sb.tile([HID, 4], f32, tag="bias32")
            nc.scalar.activation(bias32[:], tSM[0:HID, C_BENC:C_BENC + 4], Ident)
            p2B = svecBh[:, 5:10]
            beB = svecBh[:, 10:15]

            # ---- eR = relu(ea)  (DVE) ----
            eR = sb.tile([N, EDIM * N], bf, tag="eR")
            nc.vector.tensor_scalar_max(eR[:], tEA[0:N, :], 0.0)
            # W1s[:, k-block] = p1_k * W1 (one early DVE op), then one wide
            # matmul gives all five xW1s blocks; PE accumulates
            # x1T = sum_k (x @ p1_k W1)^T @ ea_k   (A1 = sum p1_k ea_k)
            W1s = sb.tile([HID, EDIM * HID], bf, tag="W1s")
            nc.vector.tensor_tensor(
                W1s[:].rearrange("p (k h) -> p k h", k=EDIM),
                W1[:, None, :].to_broadcast([HID, EDIM, HID]),
                svecBh[0:HID, 0:5][:, :, None].to_broadcast([HID, EDIM, HID]),
                A.mult,
            )

            def eRk_b(k):
                return eR[:, k * N:(k + 1) * N][:, None, :].to_broadcast(
                    [N, EDIM, N]
                )

            def WeB_b(k):
                return svecBh[:, 15 + 5 * k:20 + 5 * k][:, :, None].to_broadcast(
                    [N, EDIM, N]
                )

            # ---- mix: G3[i,m,j] = sum_k eR_k[i,j] We[k,m] ----
            # mults: k=0,1,2 DVE / k=3,4 GpSimd; add tree: D:G+=Ta, G:Tb+=Tc,
            # D:G+=Td, D:G+=Tb
            G = sb.tile([N, EDIM * N], bf, tag="G")
            G3 = G[:].rearrange("p (m j) -> p m j", m=EDIM)
            Ta = sb.tile([N, EDIM * N], bf, tag="Ta")
            Ta3 = Ta[:].rearrange("p (m j) -> p m j", m=EDIM)
            Tb = sb.tile([N, EDIM * N], bf, tag="Tb")
            Tb3 = Tb[:].rearrange("p (m j) -> p m j", m=EDIM)
            Tc = sb.tile([N, EDIM * N], bf, tag="Tc")
            Tc3 = Tc[:].rearrange("p (m j) -> p m j", m=EDIM)
            Td = sb.tile([N, EDIM * N], bf, tag="Td")
            Td3 = Td[:].rearrange("p (m j) -> p m j", m=EDIM)

            nc.vector.tensor_tensor(G3, eRk_b(0), WeB_b(0), A.mult)
            nc.vector.tensor_tensor(Ta3, eRk_b(1), WeB_b(1), A.mult)
            nc.vector.tensor_tensor(Td3, eRk_b(2), WeB_b(2), A.mult)
            nc.vector.tensor_tensor(Tb3, eRk_b(3), WeB_b(3), A.mult)
            nc.vector.tensor_tensor(Tc3, eRk_b(4), WeB_b(4), A.mult)
            nc.vector.tensor_tensor(G3, G3, Ta3, A.add)
            nc.vector.tensor_tensor(G3, G3, Td3, A.add)
            nc.gpsimd.tensor_tensor(Tb3, Tb3, Tc3, A.add)

            # ---- x side (PE + ACT), overlapped with the mix ----
            xT_ps = ps.tile([HID, N], f32, tag="ps")
            nc.tensor.matmul(xT_ps[:], Wenc, encT, start=True, stop=True)
            xT = sb.tile([HID, N], bf, tag="xT")
            nc.scalar.activation(xT[:], xT_ps[:], Ident, bias=bias32[:, 0:1])
            xW1s_ps = ps.tile([N, EDIM * HID], f32, tag="ps")
            nc.tensor.matmul(xW1s_ps[:], xT[:], W1s[:], start=True, stop=True)
            xW1s = sb.tile([N, EDIM * HID], bf, tag="xW1s")
            nc.scalar.activation(xW1s[:], xW1s_ps[:], Ident)
            x1T_ps = ps.tile([HID, N], f32, tag="ps")
            for k in range(EDIM):
                nc.tensor.matmul(
                    x1T_ps[:], xW1s[:, k * HID:(k + 1) * HID],
                    tEA[0:N, k * N:(k + 1) * N],
                    start=(k == 0), stop=(k == 4),
                )
            x1T = sb.tile([HID, N], bf, tag="x1T")
            nc.scalar.activation(x1T[:], x1T_ps[:], Relu, bias=bias32[:, 1:2])

            dvr_ps = ps.tile([1, N], f32, tag="ps")
            nc.tensor.matmul(dvr_ps[:], peT, x1T[:], start=True, stop=True)
            dvT_ps = ps.tile([N, 1], f32, tag="ps")
            nc.tensor.matmul(dvT_ps[:], x1T[:], peT, start=True, stop=True)
            # dv copies + negsum on ACT
            dv_row = sb.tile([1, N], bf, tag="dv_row")
            nc.scalar.activation(dv_row[:], dvr_ps[:], Ident)
            dvTe = sb.tile([N, 1], f32, tag="dvTe")
            nc.scalar.activation(dvTe[:], dvT_ps[:], Ident)
            dvTn = sb.tile([N, 1], f32, tag="dvTn")
            nc.scalar.activation(dvTn[:], dvT_ps[:], Ident, scale=-1.0)
            dvROW_ps = psr.tile([N, N], f32, tag="psrow")
            nc.tensor.matmul(dvROW_ps[:], ones_row[:], dv_row[:], start=True, stop=True)
            negsum = sb.tile([N, N], bf, tag="negsum")
            nc.scalar.activation(
                negsum[:], dvROW_ps[:], Ident, scale=-1.0, bias=dvTn[:, 0:1]
            )
            # xW2 early (needed only at the very end)
            xW2_ps = ps.tile([N, HID], f32, tag="ps")
            nc.tensor.matmul(xW2_ps[:], x1T[:], W2, start=True, stop=True)
            xW2 = sb.tile([N, HID], bf, tag="xW2")
            nc.scalar.activation(xW2[:], xW2_ps[:], Ident)

            # ---- colmax reciprocal: 1/(max(dv,0)+eps) per node, then
            #      crec[i,j] = min(rec_i, rec_j) (f monotone decreasing) ----
            dvm = sb.tile([N, 1], f32, tag="dvm")
            nc.vector.tensor_scalar(dvm[:], dvT_ps[:], 0.0, EPS, A.max, A.add)
            rec32 = sb.tile([N, 1], f32, tag="rec32")
            nc.vector.reciprocal(rec32[:], dvm[:])
            recb = sb.tile([N, 1], bf, tag="recb")
            nc.vector.tensor_copy(recb[:], rec32[:])
            recT_ps = ps.tile([1, N], bf, tag="ps")
            nc.tensor.transpose(recT_ps[:], recb[:], ident[:])
            rec_row = sb.tile([1, N], bf, tag="rec_row")
            nc.vector.tensor_copy(rec_row[:], recT_ps[:])
            recROW_ps = ps.tile([N, N], f32, tag="ps")
            nc.tensor.matmul(
                recROW_ps[:], ones_row[:], rec_row[:], start=True, stop=True
            )
            crecb = sb.tile([N, N], bf, tag="crecb")
            nc.vector.tensor_scalar_min(crecb[:], recROW_ps[:], rec32[:, 0:1])
            crecN = sb.tile([N, N], bf, tag="crecN")
            nc.vector.tensor_tensor(crecN[:], crecb[:], negsum[:], A.mult)

            # ---- finish the mix: single DVE add (Tb+Tc folded on GpSimd) ----
            nc.vector.tensor_tensor(G3, G3, Tb3, A.add)
            Q = sb.tile([N, EDIM * N], bf, tag="Q")
            for m in range(3):
                nc.gpsimd.tensor_tensor(
                    Q[:, m * N:(m + 1) * N], G[:, m * N:(m + 1) * N], crecN[:],
                    A.mult,
                )

            # ---- EC[i,k] = sum_j eR_k[i,j] crec[i,j]  (tt + X-reduce) ----
            ejk = sb.tile([N, EDIM * N], bf, tag="ejk")
            ejk3 = ejk[:].rearrange("p (k j) -> p k j", k=EDIM)
            nc.vector.tensor_tensor(
                ejk3, eR[:].rearrange("p (k j) -> p k j", k=EDIM),
                crecb[:, None, :].to_broadcast([N, EDIM, N]), A.mult,
            )
            EC = sb.tile([N, EDIM], bf, tag="EC")
            with nc.allow_low_precision(reason="bf16 net, 2e-2 tolerance"):
                nc.vector.tensor_reduce(
                    EC[:], ejk3, mybir.AxisListType.X, A.add
                )
            # ECd[i,k] = dv_i EC[i,k];  St[i,m] = sum_k ECd[i,k] We[k,m]
            ECd = sb.tile([N, EDIM], bf, tag="ECd")
            nc.vector.tensor_scalar_mul(ECd[:], EC[:], dvTe[:, 0:1])
            tmp3 = sb.tile([N, EDIM * EDIM], bf, tag="tmp3")
            nc.vector.tensor_tensor(
                tmp3[:].rearrange("p (m k) -> p m k", m=EDIM),
                ECd[:, None, :].to_broadcast([N, EDIM, EDIM]),
                svecBh[:, 40:65].rearrange("p (m k) -> p m k", m=EDIM),
                A.mult,
            )
            St = sb.tile([N, EDIM], bf, tag="St")
            with nc.allow_low_precision(reason="5-term sum, bf16 net"):
                nc.vector.tensor_reduce(
                    St[:], tmp3[:].rearrange("p (m k) -> p m k", m=EDIM),
                    mybir.AxisListType.X, A.add,
                )
            t1b = sb.tile([N, EDIM], f32, tag="t1b")
            nc.gpsimd.tensor_tensor(t1b[:], St[:], beB, A.add)
            for m in (3, 4):
                nc.vector.tensor_tensor(
                    Q[:, m * N:(m + 1) * N], G[:, m * N:(m + 1) * N], crecN[:],
                    A.mult,
                )

            # ---- u[(m,j)] = St[j,m] via per-column PE transposes ----
            u_psA = psr.tile([1, 3 * N], bf, tag="upsA")
            for m in range(3):
                nc.tensor.transpose(
                    u_psA[0:1, m * N:(m + 1) * N], St[:, m:m + 1], ident[:]
                )
            u_psB = psr.tile([1, 2 * N], bf, tag="upsB")
            for m in range(2):
                nc.tensor.transpose(
                    u_psB[0:1, m * N:(m + 1) * N], St[:, 3 + m:4 + m], ident[:]
                )
            u_sb = sb.tile([1, EDIM * N], bf, tag="u_sb")
            nc.scalar.activation(u_sb[0:1, 0:3 * N], u_psA[:], Ident)
            nc.vector.tensor_copy(u_sb[0:1, 3 * N:5 * N], u_psB[:])

            # ---- z = U + q accumulated on PE in two PSUM banks ----
            # bank A: planes 0-2, bank B: planes 3-4
            UA = psu.tile([N, 3 * N], f32, tag="psu")
            UB = psu.tile([N, 2 * N], f32, tag="psu")
            nc.tensor.matmul(UA[:], ident[:], Q[:, 0:3 * N],
                             start=True, stop=False)
            nc.tensor.matmul(UB[:], ident[:], Q[:, 3 * N:5 * N],
                             start=True, stop=False)
            nc.tensor.matmul(UA[:], ones_row[:], u_sb[0:1, 0:3 * N],
                             start=False, stop=True)
            nc.tensor.matmul(UB[:], ones_row[:], u_sb[0:1, 3 * N:5 * N],
                             start=False, stop=True)

            def z_m(m):
                if m < 3:
                    return UA[:, m * N:(m + 1) * N]
                return UB[:, (m - 3) * N:(m - 2) * N]

            # ---- accm_m = rowsum(relu(z_m + t1b_m)): 2 ACT + 3 DVE ----
            accmA = sb.tile([N, 2], f32, tag="accmA")
            accmD = sb.tile([N, 3], f32, tag="accmD")
            for m in (0, 1):
                R = pm.tile([N, N], bf, tag="R")
                nc.scalar.activation(
                    R[:], z_m(m), Relu, bias=t1b[:, m:m + 1],
                    accum_out=accmA[:, m:m + 1],
                )
            # DVE takes bank-B planes first (no shared PSUM tile with the
            # ACT readers of bank A -> no cross-engine serialization)
            for m in (3, 4, 2):
                R = pm.tile([N, N], bf, tag="R")
                nc.vector.scalar_tensor_tensor(
                    R[:], z_m(m), t1b[:, m:m + 1], zerosT[:], A.add, A.max,
                    accum_out=accmD[:, m - 2:m - 1],
                )

            # ---- diagonal correction (GpSimd + DVE, tiny) ----
            darg = sb.tile([N, EDIM], bf, tag="darg")
            nc.gpsimd.tensor_tensor(darg[:], St[:], t1b[:], A.add)
            dr = sb.tile([N, EDIM], bf, tag="dr")
            nc.vector.tensor_scalar_max(dr[:], darg[:], 0.0)
            ds = sb.tile([N, 1], f32, tag="ds")
            dscr = sb.tile([N, EDIM], bf, tag="dscr")
            nc.vector.scalar_tensor_tensor(
                dscr[:], dr[:], 1.0, p2B, A.mult, A.mult, accum_out=ds[:]
            )

            # ---- s = ((sum_m p2_m accm_m) - diag)/N; pooled head ----
            sA = sb.tile([N, 1], f32, tag="sA")
            sAscr = sb.tile([N, 2], bf, tag="sAscr")
            nc.vector.scalar_tensor_tensor(
                sAscr[:], accmA[:], 1.0, p2B[:, 0:2], A.mult, A.mult,
                accum_out=sA[:],
            )
            sD = sb.tile([N, 1], f32, tag="sD")
            sDscr = sb.tile([N, 3], bf, tag="sDscr")
            nc.vector.scalar_tensor_tensor(
                sDscr[:], accmD[:], 1.0, p2B[:, 2:5], A.mult, A.mult,
                accum_out=sD[:],
            )
            s0 = sb.tile([N, 1], f32, tag="s0")
            nc.vector.tensor_tensor(s0[:], sA[:], sD[:], A.add)
            sfin = sb.tile([N, 1], bf, tag="sfin")
            nc.vector.tensor_scalar(
                sfin[:], s0[:], ds[:, 0:1], 1.0 / N, A.subtract, A.mult
            )
            pooled_ps = ps.tile([HID, 1], f32, tag="ps")
            nc.tensor.matmul(pooled_ps[:], xW2[:], sfin[:], start=True, stop=True)
            nc.vector.tensor_scalar_add(
                poolT[0:HID, 0:1], pooled_ps[:], bias32[:, 2:3]
            )
            outT_ps = ps.tile([OUT, 1], f32, tag="ps")
            nc.tensor.matmul(outT_ps[:], WlA, poolT[:], start=True, stop=True)
            out_sb = sb.tile([OUT, 1], f32, tag="out_sb")
            nc.vector.tensor_copy(out_sb[:], outT_ps[:])
            nc.sync.dma_start(out=out_d[:], in_=out_sb[:])

    from concourse.library_overlay import lower_extended_insts

    lower_extended_insts(nc)
    _split_excess_waits(nc, mybir)
    return nc


def _prep_inputs(inputs):
    from ml_dtypes import bfloat16

    ei = np.asarray(inputs["edge_index"][0], dtype=np.int64)
    ej = np.asarray(inputs["edge_index"][1], dtype=np.int64)
    ea = np.asarray(inputs["edge_attr"], dtype=np.float32)

    ead = np.zeros((N, EDIM, N), dtype=np.float32)
    ead[ei, :, ej] = ea
    ead[ej, :, ei] = ea
    ead = ead.reshape(N, EDIM * N)

    smalls = np.zeros((128, SM_W), dtype=np.float32)
    smalls[0:HID, C_W1:C_W1 + HID] = np.asarray(inputs["W1"], np.float32)
    smalls[0:HID, C_W2:C_W2 + HID] = np.asarray(inputs["W2"], np.float32)
    smalls[0:HID, C_WL:C_WL + OUT] = np.asarray(inputs["Wl"], np.float32)
    smalls[HID, C_WL:C_WL + OUT] = np.asarray(inputs["bl"], np.float32).reshape(-1)
    smalls[0:HID, C_PET] = np.asarray(inputs["pe"], np.float32).reshape(-1)
    smalls[0:HID, C_BENC] = np.asarray(inputs["b_enc"], np.float32).reshape(-1)
    smalls[0:HID, C_B1] = np.asarray(inputs["b1"], np.float32).reshape(-1)
    smalls[0:HID, C_B2] = np.asarray(inputs["b2"], np.float32).reshape(-1)
    smalls[0:OUT, C_BL] = np.asarray(inputs["bl"], np.float32).reshape(-1)
    We = np.asarray(inputs["We"], np.float32)
    svec = np.concatenate(
        [
            np.asarray(inputs["p1"], np.float32).reshape(-1),
            np.asarray(inputs["p2"], np.float32).reshape(-1),
            np.asarray(inputs["be"], np.float32).reshape(-1),
            We.reshape(-1),          # k-major: col 15+5k+m
            We.T.reshape(-1),        # m-major: col 40+5m+k
        ]
    )
    # svec shipped pre-broadcast in smv instead

    encw = np.zeros((ENC, N + HID), dtype=np.float32)
    encw[:, 0:N] = np.asarray(inputs["encoding_raw"], np.float32).T
    encw[:, N:N + HID] = np.asarray(inputs["W_enc"], np.float32)

    def pad_rows(a):
        out = np.zeros((128, a.shape[1]), dtype=np.float32)
        out[0:a.shape[0]] = a
        return out

    def pad128(a):
        out = np.zeros((128, a.shape[1]), dtype=np.float32)
        out[0:a.shape[0]] = a
        return out.astype(bfloat16)

    svb = np.zeros((128, 72), dtype=np.float32)
    svb[0:N, 0:65] = svec[None, :]
    svb[0:EDIM, 66:71] = We
    smv = np.concatenate([smalls[0:128], svb], axis=1)
    eadp = np.zeros((128, EDIM * N), dtype=np.float32)
    eadp[0:N] = ead
    return {
        "eaT": eadp[0:64].astype(bfloat16),
        "eaB": eadp[64:128].astype(bfloat16),
        "smv": smv.astype(bfloat16),
        "encw": pad_rows(encw).astype(bfloat16),
    }


def kernel(**inputs) -> np.ndarray:
    import sys

    if "/opt/trn_rl_repo" not in sys.path:
        sys.path.insert(0, "/opt/trn_rl_repo")
    from concourse.bass_utils import run_bass_kernel_spmd

    if "nc" not in _CACHE:
        _CACHE["nc"] = _build()
    nc = _CACHE["nc"]

    in_map = _prep_inputs(inputs)
    res = run_bass_kernel_spmd(
        nc, [in_map] * 8, core_ids=list(range(8)), trace=False
    )
    return np.asarray(res.results[0]["out"], dtype=np.float32).reshape(1, OUT)
